# revision 8
# baseline (speedup 1.0000x reference)
"""Dense transformer (ChatGenerator) forward on 8 TRN2 NeuronCores.

Sharding: data-parallel over (batch, seq-half) -> 8 shards of 256 tokens.
Core c handles batch c//2, tokens [(c%2)*256, (c%2+1)*256). Weights are
replicated per core in bf16. Cross-core traffic: a pairwise AllGather of
each half's K/V per layer, and one 8-way AllGather of the final-LN
activations feeding a vocab-sharded tied lm_head (each core computes all
2048 tokens x a 4096-wide vocab slice; host concatenates).

Layout: activations feature-major ("T": [D on partitions, tokens free]) so
matmuls chain without PE transposes. LayerNorm stats and softmax sums use
ones-vector matmuls. Attention is software-pipelined across heads: head
i+1's score matmuls are emitted before head i's AV so the PE never waits
on the exp/mask stages; the softmax denominator comes free from a ones
column appended to V (AV psum row 64), and normalization happens after AV
via one K=2 broadcast matmul per head pair.

Precision: weights/activations bf16 into the PE (fp32 PSUM accumulate),
residual stream fp32 in SBUF. LN scales folded into the following weight
matrices host-side; biases in this model are all zero and skipped.
Logits emitted bf16.
"""
import sys

sys.path.insert(0, "/opt/trn_rl_repo")

import math
import numpy as np
import ml_dtypes
from contextlib import ExitStack

import concourse.bass as bass
import concourse.tile as tile
from concourse import mybir
from concourse.bass_utils import run_bass_kernel_spmd

P = 128
F32 = mybir.dt.float32
BF16 = mybir.dt.bfloat16
AF = mybir.ActivationFunctionType
OP = mybir.AluOpType

DEFAULT_CFG = dict(L=6, D=1024, T=256, FF=4096, V=32000, VP=32768, H=16,
                   DK=64, S=512, B=4)


def split_sync_waits(nc, maxw=1):
    """walrus in this container rejects >1 sync-wait command per instruction;
    move excess waits onto sequencer nops inserted just before."""
    n_split = 0
    uid = 0
    for fn in nc.m.functions:
        for bb in fn.blocks:
            il = bb.instructions
            i = 0
            while i < len(il):
                inst = il[i]
                si = inst.sync_info
                waits = list(si.on_wait) if si is not None and si.on_wait else []
                if len(waits) > maxw:
                    extra, keep = waits[:-maxw], waits[-maxw:]
                    pos = i
                    for j in range(0, len(extra), maxw):
                        nop = mybir.InstNoOp(
                            name=f"waitsplit-{uid}",
                            engine=inst.engine,
                            ins=[],
                            outs=[],
                            sync_info=mybir.SyncInfo(
                                on_wait=extra[j:j + maxw], on_update=[]),
                        )
                        uid += 1
                        il.insert(pos, nop)
                        pos += 1
                        i += 1
                    inst.sync_info = mybir.SyncInfo(
                        on_wait=keep,
                        on_update=list(si.on_update) if si.on_update else [])
                    n_split += 1
                i += 1
    return n_split


def build(cfg=DEFAULT_CFG):
    L, D, T, FF = cfg["L"], cfg["D"], cfg["T"], cfg["FF"]
    H, DK, S = cfg["H"], cfg["DK"], cfg["S"]
    VP = cfg["VP"]
    VSH = VP // 8            # vocab shard per core
    VBS = 512                # vocab block (matmul moving free dim)
    NVB = VSH // VBS
    DCH = D // P             # feature chunks (8)
    TT = T // P              # token tiles per core (2)
    NKT = S // P             # key tiles over full sequence (4)
    HPT = P // DK            # heads per 128-row tile (2)
    FFC = 8                  # ff chunks per ffn block
    NB = FF // (P * FFC)     # ffn blocks (4)
    VE = DK + 1              # v columns per head incl. ones column
    KV_SZ = D * T + P * TT * H * VE
    NSH = 8                  # token shards (cores)
    scale = 1.0 / math.sqrt(DK)

    no_attn = cfg.get("no_attn", False)
    no_ffn = cfg.get("no_ffn", False)
    nc = bass.Bass()
    x0T_p = nc.declare_dram_parameter("x0T", [D, T], F32, isOutput=False)
    wq_p = nc.declare_dram_parameter("wq", [L, D, D], BF16, isOutput=False)
    wk_p = nc.declare_dram_parameter("wk", [L, D, D], BF16, isOutput=False)
    wv_p = nc.declare_dram_parameter("wv", [L, D, D], BF16, isOutput=False)
    wo_p = nc.declare_dram_parameter("wo", [L, D, D], BF16, isOutput=False)
    w1_p = nc.declare_dram_parameter("w1r", [L, NB, D, P * FFC], BF16, isOutput=False)
    w2_p = nc.declare_dram_parameter("w2r", [L, NB, P * FFC, D], BF16, isOutput=False)
    embTs_p = nc.declare_dram_parameter("embTs", [D, VSH], BF16, isOutput=False)
    mask_p = nc.declare_dram_parameter("mask", [NKT, P, T], BF16, isOutput=False)
    out_p = nc.declare_dram_parameter("logits", [NSH * T, VSH], BF16, isOutput=True)

    with tile.TileContext(nc) as tc, ExitStack() as ctx:
        pers = ctx.enter_context(tc.tile_pool(name="pers", bufs=1))
        dram = ctx.enter_context(tc.tile_pool(name="dram", bufs=1, space="DRAM"))

        # persistent state
        xT = pers.tile([P, DCH, T], F32, name="xT")
        nc.sync.dma_start(out=xT, in_=x0T_p[:].rearrange("(po pi) t -> pi po t", pi=P))
        masks_t = pers.tile([P, NKT, T], BF16, name="masks_t")
        nc.sync.dma_start(out=masks_t, in_=mask_p[:].rearrange("k p t -> p k t"))
        ones_bf = pers.tile([P, P], BF16, name="ones_bf")
        nc.vector.memset(ones_bf, 1.0)
        ones_col = ones_bf[:, 0:1]      # [128,1] partition-sum lhsT
        ones_row = ones_bf[0:1, :]      # [1,128] broadcast lhsT
        eps_t = pers.tile([P, 1], F32, name="eps_t")
        nc.vector.memset(eps_t, 1e-5)
        hf_snd = dram.tile([D * T], BF16, name="hfsnd")
        hf_ag = dram.tile([NSH, D * T], BF16, name="hfag")

        with ExitStack() as lctx:
            wp = lctx.enter_context(tc.tile_pool(name="wp", bufs=1))
            sb = lctx.enter_context(tc.tile_pool(name="sb", bufs=1))
            ps = lctx.enter_context(tc.tile_pool(name="ps", bufs=1, space="PSUM"))

            def layernorm(name):
                """h = (x - mean) * rstd over feature dim (partition axis), bf16."""
                xbf = sb.tile([P, DCH, T], BF16, tag="xbf", name=f"xbf{name}", bufs=1)
                sq = sb.tile([P, DCH, T], BF16, tag="sq", name=f"sq{name}", bufs=1)
                for d in range(DCH):
                    nc.vector.tensor_copy(out=xbf[:, d, :], in_=xT[:, d, :])
                    nc.vector.tensor_mul(sq[:, d, :], xbf[:, d, :], xbf[:, d, :])
                lnsq = ps.tile([33, T], F32, tag="aux", name=f"lnsq{name}", bufs=1)
                ps_s, ps_q = lnsq[0:1, :], lnsq[32:33, :]
                for d in range(DCH):
                    nc.tensor.matmul(ps_s, ones_col, xbf[:, d, :],
                                     start=(d == 0), stop=(d == DCH - 1))
                for d in range(DCH):
                    nc.tensor.matmul(ps_q, ones_col, sq[:, d, :],
                                     start=(d == 0), stop=(d == DCH - 1))
                vec = sb.tile([1, 2 * T], F32, tag="lnvec", name=f"lnvec{name}", bufs=1)
                tmp = sb.tile([1, 2 * T], F32, tag="lntmp", name=f"lntmp{name}", bufs=1)
                negm, rstd = vec[:, 0:T], vec[:, T:2 * T]
                nc.vector.tensor_scalar_mul(negm, ps_s, -1.0 / D)
                nc.vector.tensor_scalar_mul(tmp[:, 0:T], ps_q, 1.0 / D)
                nc.vector.tensor_mul(tmp[:, T:2 * T], negm, negm)
                nc.vector.tensor_sub(tmp[:, 0:T], tmp[:, 0:T], tmp[:, T:2 * T])
                nc.scalar.activation(out=tmp[:, T:2 * T], in_=tmp[:, 0:T],
                                     func=AF.Sqrt, bias=eps_t[0:1, :])
                nc.vector.reciprocal(rstd, tmp[:, T:2 * T])
                vecbf = sb.tile([1, 2 * T], BF16, tag="lnvecbf", name=f"lnvb{name}", bufs=1)
                nc.vector.tensor_copy(out=vecbf, in_=vec)
                bc = ps.tile([P, 2 * T], F32, tag="bc", name=f"lnbc{name}", bufs=1)
                nc.tensor.matmul(bc, ones_row, vecbf, start=True, stop=True)
                h = sb.tile([P, DCH, T], BF16, tag="hT", name=f"h{name}", bufs=1)
                for d in range(DCH):
                    ta = sb.tile([P, T], F32, tag="lnapply", name=f"lna{name}_{d}", bufs=2)
                    nc.vector.tensor_tensor(ta, xT[:, d, :], bc[:, 0:T], OP.add)
                    nc.vector.tensor_tensor(h[:, d, :], ta, bc[:, T:2 * T], OP.mult)
                return h

            for l in range(L):
                h1 = layernorm(f"1_{l}")

                if not no_attn:
                    # ---- K projection (feature-major) ----
                    wk_s = wp.tile([P, DCH, D], BF16, tag="wqkvo", name=f"wk{l}", bufs=2)
                    nc.sync.dma_start(out=wk_s, in_=wk_p[:][l].rearrange("(po pi) n -> pi po n", pi=P))
                    kT = sb.tile([P, DCH, T], BF16, tag="kT", name=f"kT{l}", bufs=1)
                    for m in range(DCH):
                        pk = ps.tile([P, T], F32, tag="mm", name=f"pk{l}_{m}", bufs=2)
                        for k in range(DCH):
                            nc.tensor.matmul(pk, wk_s[:, k, m * P:(m + 1) * P], h1[:, k, :],
                                             start=(k == 0), stop=(k == DCH - 1))
                        nc.vector.tensor_copy(out=kT[:, m, :], in_=pk)

                    # ---- V projection (token-major, ones col at DK per head) ----
                    wv_s = wp.tile([P, DCH, D], BF16, tag="wqkvo", name=f"wv{l}", bufs=2)
                    nc.sync.dma_start(out=wv_s, in_=wv_p[:][l].rearrange("(po pi) n -> pi po n", pi=P))
                    vx = sb.tile([P, TT, H, VE], BF16, tag="vx", name=f"vx{l}", bufs=1)
                    nc.vector.memset(vx[:, :, :, DK:VE], 1.0)
                    for t2 in range(TT):
                        for nh in range(D // 512):
                            pv = ps.tile([P, 512], F32, tag="mm", name=f"pv{l}_{t2}_{nh}", bufs=2)
                            for k in range(DCH):
                                nc.tensor.matmul(pv, h1[:, k, t2 * P:(t2 + 1) * P],
                                                 wv_s[:, k, nh * 512:(nh + 1) * 512],
                                                 start=(k == 0), stop=(k == DCH - 1))
                            nc.vector.tensor_copy(
                                out=vx[:, t2, nh * 8:(nh + 1) * 8, 0:DK],
                                in_=pv.rearrange("p (h e) -> p h e", e=DK))

                    # ---- pairwise allgather of k/v ----
                    kv_snd = dram.tile([KV_SZ], BF16, tag="kvs", name=f"kvs{l}", bufs=1)
                    kv_ag = dram.tile([2, KV_SZ], BF16, tag="kva", name=f"kva{l}", bufs=1)
                    nc.sync.dma_start(
                        out=kv_snd[0:D * T].rearrange("(pi po t) -> pi po t", po=DCH, t=T),
                        in_=kT)
                    nc.sync.dma_start(
                        out=kv_snd[D * T:].rearrange("(pi tt h e) -> pi tt h e",
                                                     tt=TT, h=H, e=VE),
                        in_=vx)
                    if not cfg.get("no_ag", False):
                        nc.gpsimd.collective_compute(
                            "AllGather", OP.bypass,
                            ins=[kv_snd.opt()], outs=[kv_ag.opt()],
                            replica_groups=[[0, 1], [2, 3], [4, 5], [6, 7]],
                        )
                    else:
                        nc.gpsimd.dma_start(out=kv_ag[0], in_=kv_snd)
                        nc.gpsimd.dma_start(out=kv_ag[1], in_=kv_snd)

                    # q while the allgather is in flight
                    wq_s = wp.tile([P, DCH, D], BF16, tag="wqkvo", name=f"wq{l}", bufs=2)
                    nc.sync.dma_start(out=wq_s, in_=wq_p[:][l].rearrange("(po pi) n -> pi po n", pi=P))
                    qT = sb.tile([P, DCH, T], BF16, tag="qT", name=f"qT{l}", bufs=1)
                    for m in range(DCH):
                        pq = ps.tile([P, T], F32, tag="mm", name=f"pq{l}_{m}", bufs=2)
                        for k in range(DCH):
                            nc.tensor.matmul(pq, wq_s[:, k, m * P:(m + 1) * P], h1[:, k, :],
                                             start=(k == 0), stop=(k == DCH - 1))
                        nc.vector.tensor_copy(out=qT[:, m, :], in_=pq)

                    # canonical kt order from the gathered buffer (uniform on
                    # both pair members: src 0 = even rank = seq first half)
                    kTs = sb.tile([P, 2, DCH, T], BF16, tag="kts", name=f"kts{l}", bufs=1)
                    vxs = sb.tile([P, 2, TT, H, VE], BF16, tag="vxs", name=f"vxs{l}", bufs=1)
                    for src in range(2):
                        nc.sync.dma_start(
                            out=kTs[:, src],
                            in_=kv_ag[src, 0:D * T].rearrange(
                                "(pi po t) -> pi po t", po=DCH, t=T))
                        nc.sync.dma_start(
                            out=vxs[:, src],
                            in_=kv_ag[src, D * T:].rearrange(
                                "(pi tt h e) -> pi tt h e", tt=TT, h=H, e=VE))

                    # ---- attention, software-pipelined across heads ----
                    oT = sb.tile([P, DCH, T], BF16, tag="oT", name=f"oT{l}", bufs=1)
                    zbp = {}

                    def stage1(hd):
                        pt, sub = hd // HPT, hd % HPT
                        pb = sub * DK
                        expT = sb.tile([P, NKT, T], BF16, tag="expT",
                                       name=f"expT{l}_{hd}", bufs=3)
                        for kt in range(NKT):
                            src, t2 = kt // TT, kt % TT
                            if kt % 2 == 0:
                                sc2 = ps.tile([P, 2 * T], F32, tag="sc",
                                              name=f"sc{l}_{hd}_{kt}", bufs=2)
                            sc = sc2[:, (kt % 2) * T:(kt % 2 + 1) * T]
                            nc.tensor.matmul(
                                sc,
                                kTs[pb:pb + DK, src, pt, t2 * P:(t2 + 1) * P],
                                qT[pb:pb + DK, pt, :], start=True, stop=True)
                            et = sb.tile([P, T], BF16, tag="et",
                                         name=f"et{l}_{hd}_{kt}", bufs=8)
                            nc.scalar.activation(out=et, in_=sc, func=AF.Exp, scale=scale)
                            nc.vector.tensor_tensor(expT[:, kt, :], et,
                                                    masks_t[:, kt, :], OP.mult)
                        return expT

                    po_pair = {}

                    def stage2(hd, expT):
                        pt, sub = hd // HPT, hd % HPT
                        if sub == 0:
                            po_pair[pt] = ps.tile([VE, 2 * T], F32, tag="po",
                                                  name=f"po{l}_{pt}", bufs=2)
                        po = po_pair[pt][:, sub * T:(sub + 1) * T]
                        for kt in range(NKT):
                            src, t2 = kt // TT, kt % TT
                            nc.tensor.matmul(po, vxs[:, src, t2, hd, :],
                                             expT[:, kt, :],
                                             start=(kt == 0), stop=(kt == NKT - 1))
                        zrecf = sb.tile([1, T], F32, tag="zrecf",
                                        name=f"zrf{l}_{hd}", bufs=4)
                        nc.vector.reciprocal(zrecf, po[DK:VE, :])
                        zrec_b = sb.tile([1, T], BF16, tag="zrecb",
                                         name=f"zrb{l}_{hd}", bufs=4)
                        nc.vector.tensor_copy(out=zrec_b, in_=zrecf)
                        if sub == 0:
                            zbp[pt] = ps.tile([P, T], F32, tag="bc",
                                              name=f"zb{l}_{pt}", bufs=1)
                        nc.tensor.matmul(zbp[pt][sub * DK:(sub + 1) * DK, :],
                                         ones_row[:, 0:DK], zrec_b,
                                         start=True, stop=True)

                    def stage3(pt):
                        zbs = sb.tile([P, T], BF16, tag="zbs", name=f"zbs{l}_{pt}", bufs=2)
                        nc.vector.tensor_copy(out=zbs, in_=zbp.pop(pt))
                        pp = po_pair.pop(pt)
                        nc.vector.tensor_tensor(oT[0:DK, pt, :], pp[0:DK, 0:T],
                                                zbs[0:DK, :], OP.mult)
                        nc.vector.tensor_tensor(oT[DK:P, pt, :], pp[0:DK, T:2 * T],
                                                zbs[DK:P, :], OP.mult)

                    def drain(hd, expT):
                        pt, sub = hd // HPT, hd % HPT
                        stage2(hd, expT)
                        if sub == 1:
                            stage3(pt)

                    pend = []
                    for hd in range(H):
                        e = stage1(hd)
                        pend.append((hd, e))
                        if len(pend) > 2:
                            drain(*pend.pop(0))
                    for item in pend:
                        drain(*item)

                    # ---- Wo + residual ----
                    wo_s = wp.tile([P, DCH, D], BF16, tag="wqkvo", name=f"wo{l}", bufs=2)
                    nc.sync.dma_start(out=wo_s, in_=wo_p[:][l].rearrange("(po pi) n -> pi po n", pi=P))
                    for m in range(DCH):
                        pso = ps.tile([P, T], F32, tag="mm", name=f"pso{l}_{m}", bufs=2)
                        for k in range(DCH):
                            nc.tensor.matmul(pso, wo_s[:, k, m * P:(m + 1) * P], oT[:, k, :],
                                             start=(k == 0), stop=(k == DCH - 1))
                        nc.vector.tensor_tensor(xT[:, m, :], xT[:, m, :], pso, OP.add)

                # ---- FFN in blocks of 1024 ff dims ----
                if not no_ffn:
                    h2 = layernorm(f"2_{l}")
                    for b in range(NB):
                        w1_b = wp.tile([P, DCH, P * FFC], BF16, tag="w1", name=f"w1_{l}_{b}", bufs=2)
                        nc.sync.dma_start(
                            out=w1_b,
                            in_=w1_p[:][l, b].rearrange("(po pi) n -> pi po n", pi=P))
                        w2_b = wp.tile([P, FFC, D], BF16, tag="w2", name=f"w2_{l}_{b}", bufs=2)
                        nc.sync.dma_start(
                            out=w2_b,
                            in_=w2_p[:][l, b].rearrange("(po pi) n -> pi po n", pi=P))
                        gT = sb.tile([P, FFC, T], BF16, tag="gT", name=f"gT{l}_{b}", bufs=2)
                        for f in range(FFC):
                            psg = ps.tile([P, T], F32, tag="mm", name=f"psg{l}_{b}_{f}", bufs=2)
                            for k in range(DCH):
                                nc.tensor.matmul(psg, w1_b[:, k, f * P:(f + 1) * P], h2[:, k, :],
                                                 start=(k == 0), stop=(k == DCH - 1))
                            nc.scalar.activation(out=gT[:, f, :], in_=psg, func=AF.Gelu)
                        for m in range(DCH):
                            psf = ps.tile([P, T], F32, tag="mm", name=f"psf{l}_{b}_{m}", bufs=2)
                            for f in range(FFC):
                                nc.tensor.matmul(psf, w2_b[:, f, m * P:(m + 1) * P], gT[:, f, :],
                                                 start=(f == 0), stop=(f == FFC - 1))
                            nc.vector.tensor_tensor(xT[:, m, :], xT[:, m, :], psf, OP.add)

            # ---- final LN, ship activations for the vocab-sharded head ----
            hf = layernorm("f")
            nc.sync.dma_start(
                out=hf_snd[:].rearrange("(pi po t) -> pi po t", po=DCH, t=T),
                in_=hf)
            if not cfg.get("no_ag", False):
                nc.gpsimd.collective_compute(
                    "AllGather", OP.bypass,
                    ins=[hf_snd.opt()], outs=[hf_ag.opt()],
                    replica_groups=[[0, 1, 2, 3, 4, 5, 6, 7]],
                )
            else:
                for s in range(NSH):
                    nc.gpsimd.dma_start(out=hf_ag[s], in_=hf_snd)

        # ---- tied lm head over this core's vocab shard ----
        with ExitStack() as hctx:
            hp = hctx.enter_context(tc.tile_pool(name="hp", bufs=1))
            ps2 = hctx.enter_context(tc.tile_pool(name="ps2", bufs=1, space="PSUM"))
            hf_all = hp.tile([P, NSH, DCH, T], BF16, name="hfall")
            nc.sync.dma_start(
                out=hf_all,
                in_=hf_ag[:].rearrange("s (pi po t) -> pi s po t", po=DCH, t=T))
            for vb in range(NVB):
                et = hp.tile([P, DCH, VBS], BF16, tag="emb", name=f"emb{vb}", bufs=2)
                nc.sync.dma_start(
                    out=et,
                    in_=embTs_p[:][:, vb * VBS:(vb + 1) * VBS].rearrange(
                        "(po pi) v -> pi po v", pi=P))
                for s in range(NSH):
                    for t2 in range(TT):
                        pl = ps2.tile([P, VBS], F32, tag="pl", name=f"pl{vb}_{s}_{t2}", bufs=3)
                        for k in range(DCH):
                            nc.tensor.matmul(pl, hf_all[:, s, k, t2 * P:(t2 + 1) * P],
                                             et[:, k, :],
                                             start=(k == 0), stop=(k == DCH - 1))
                        st = hp.tile([P, VBS], BF16, tag="ostage", name=f"st{vb}_{s}_{t2}", bufs=3)
                        nc.vector.tensor_copy(out=st, in_=pl)
                        nc.sync.dma_start(
                            out=out_p[:][s * T + t2 * P:s * T + (t2 + 1) * P,
                                         vb * VBS:(vb + 1) * VBS],
                            in_=st)

    split_sync_waits(nc, maxw=1)
    return nc


def prep_inputs(inputs, cfg=DEFAULT_CFG):
    """Host-side: embedding gather + pe add, LN-scale folding, bf16 casts,
    transposes, per-core sharding, causal masks, vocab-shard slices."""
    L, D, T, FF = cfg["L"], cfg["D"], cfg["T"], cfg["FF"]
    S, B, V, VP = cfg["S"], cfg["B"], cfg["V"], cfg["VP"]
    NKT = S // P
    FFC = 8
    NB = FF // (P * FFC)
    VSH = VP // 8
    bf = ml_dtypes.bfloat16

    ids = np.asarray(inputs["input_ids"])
    emb = np.asarray(inputs["emb"], np.float32)
    pe = np.asarray(inputs["pe"], np.float32)
    ln1_s = np.asarray(inputs["ln1_s"], np.float32)
    ln2_s = np.asarray(inputs["ln2_s"], np.float32)
    lnf_s = np.asarray(inputs["lnf_s"], np.float32)

    # x0 = emb[ids] + pe[batch_index] (faithful quirk: pe row by batch index)
    x0 = emb[ids]                                   # [B,S,D]
    x0 = x0 + pe[:B][:, None, :]

    wq = np.asarray(inputs["Wq"], np.float32) * ln1_s[:, :, None]
    wk = np.asarray(inputs["Wk"], np.float32) * ln1_s[:, :, None]
    wv = np.asarray(inputs["Wv"], np.float32) * ln1_s[:, :, None]
    wo = np.asarray(inputs["Wo"], np.float32)
    w1 = np.asarray(inputs["W1"], np.float32) * ln2_s[:, :, None]
    w2 = np.asarray(inputs["W2"], np.float32)
    embT = np.zeros((D, VP), np.float32)
    embT[:, :V] = (emb[:V] * lnf_s[None, :]).T      # [D, VP] padded

    wq_b = wq[:L].astype(bf)
    wk_b = wk[:L].astype(bf)
    wv_b = wv[:L].astype(bf)
    wo_b = wo[:L].astype(bf)
    w1r = np.ascontiguousarray(
        w1[:L].reshape(L, D, NB, P * FFC).transpose(0, 2, 1, 3)).astype(bf)
    w2r = np.ascontiguousarray(
        w2[:L].reshape(L, NB, P * FFC, D)).astype(bf)
    embT_b = embT.astype(bf)

    ff = np.arange(T)[None, :]
    pp = np.arange(P)[:, None]
    tri0 = (pp <= ff).astype(np.float32)
    tri1 = (P + pp <= ff).astype(np.float32)

    in_maps = []
    for c in range(8):
        b, h = c // 2, c % 2
        xt = np.ascontiguousarray(x0[b, h * T:(h + 1) * T, :].T).astype(np.float32)
        mask = np.zeros((NKT, P, T), np.float32)
        if h == 0:
            mask[0], mask[1] = tri0, tri1           # kt 2,3 fully masked
        else:
            mask[0], mask[1] = 1.0, 1.0             # first half fully visible
            mask[2], mask[3] = tri0, tri1
        in_maps.append({
            "x0T": xt,
            "wq": wq_b, "wk": wk_b, "wv": wv_b, "wo": wo_b,
            "w1r": w1r, "w2r": w2r,
            "embTs": np.ascontiguousarray(embT_b[:, c * VSH:(c + 1) * VSH]),
            "mask": mask.astype(bf),
        })
    return in_maps


def assemble(outs, cfg=DEFAULT_CFG):
    """outs[c] = [8*T, VSH] bf16 logits (rows by token shard, cols = core c's
    vocab slice). Returns [B, S, V] fp32."""
    B, S, T, V, VP = cfg["B"], cfg["S"], cfg["T"], cfg["V"], cfg["VP"]
    VSH = VP // 8
    full = np.empty((B, S, VP), np.float32)
    for c in range(8):
        blk = np.asarray(outs[c], dtype=np.float32)  # [2048, VSH]
        for s in range(8):
            sb_, sh = s // 2, s % 2
            full[sb_, sh * T:(sh + 1) * T, c * VSH:(c + 1) * VSH] = \
                blk[s * T:(s + 1) * T]
    return full[:, :, :V]


_BUILT = {}


def run_cores(in_maps, cfg=DEFAULT_CFG):
    key = tuple(sorted((k, v) for k, v in cfg.items()))
    if key not in _BUILT:
        _BUILT[key] = build(cfg)
    nc = _BUILT[key]
    res = run_bass_kernel_spmd(nc, in_maps, core_ids=list(range(8)))
    return [r["logits"] for r in res.results]


def kernel(**inputs):
    cfg = DEFAULT_CFG
    in_maps = prep_inputs(inputs, cfg)
    outs = run_cores(in_maps, cfg)
    return assemble(outs, cfg)
